# revision 1
# baseline (speedup 1.0000x reference)
"""BatchHardTripletLoss on 8 TRN2 NeuronCores (Bass/Tile).

The warm-path cost of this problem is host<->device traffic over the axon
tunnel (~60 MB/s up, ~30 MB/s down), not on-chip compute, so the kernel is
built around moving as few bytes as possible:

  - Host: sort rows by label, pad every class segment to SEG=1024 rows with
    far-away dummy rows (first coord DUMMY_VAL -> sq ~ 1e6, so dummies never
    win a hardest-negative).  Core i uploads ONLY its anchor shard: tile i of
    every class, transposed, in fp16 ([128, 1280] = 327 KB), plus a 5 KB
    anchor-validity mask.  Total upload ~2.7 MB instead of ~105 MB.
  - On chip: an AllGather over NeuronLink reassembles the full padded
    embedding matrix (the 8 anchor shards tile it exactly).  Squared norms
    are computed on chip (square + ones-matmul partition reduction), the
    distance-matrix sweep accumulates hardest-pos/neg in "2*dot - sq_j"
    space via DVE scalar_tensor_tensor off PSUM, and a PE-transpose fold
    reduces everything to one f32 loss partial per core ([1,1] download).
  - Host folds 8 scalars.
"""

import numpy as np

import jax

# Cache compiled XLA executables on disk: run_bass_kernel_spmd builds a fresh
# jax.jit closure per call, so without this every warm call pays a ~165 ms
# recompile before dispatch.
try:
    jax.config.update("jax_compilation_cache_dir", "/tmp/jax_comp_cache")
    jax.config.update("jax_persistent_cache_min_compile_time_secs", 0.0)
    jax.config.update("jax_persistent_cache_min_entry_size_bytes", 0)
except Exception:
    pass

import concourse.bass as bass
import concourse.bacc as bacc
import concourse.tile as tile
from concourse import masks, mybir
from concourse.bass_utils import run_bass_kernel_spmd

B, D, NCLASS = 8192, 128, 10
SEG = 1024                 # padded rows per class
TPC = SEG // 128           # 128-row tiles per class = 8
NCORES = 8
BPAD = NCLASS * SEG        # 10240
NJT = BPAD // 128          # 80 j-tiles
NA = NCLASS * 128          # anchors per core = 1280
F32 = mybir.dt.float32
F16 = mybir.dt.float16
AFT = mybir.ActivationFunctionType
ALU = mybir.AluOpType
MARGIN = 1.0
DUMMY_VAL = 1000.0


def build_nc(R, gather_addr_space="Shared"):
    """R: real row count per class (0 <= R[k] <= SEG)."""
    nc = bacc.Bacc()
    esh_d = nc.dram_tensor("esh", [D, NA], F16, kind="ExternalInput")
    amask_d = nc.dram_tensor("amask", [128, NCLASS], F32, kind="ExternalInput")
    loss_d = nc.dram_tensor("loss", [1, 1], F32, kind="ExternalOutput")

    with tile.TileContext(nc) as tc:
        with (
            tc.tile_pool(name="sb", bufs=1) as sb,
            tc.tile_pool(name="dram", bufs=1, space="DRAM") as dram,
        ):
            # ---- kick off the AllGather first: cores exchange anchor shards
            # so each one can rebuild the full [128, 10240] embedding matrix.
            bounce = dram.tile([D, NA], F16, tag="bounce")
            gath = dram.tile([NCORES * D, NA], F16, tag="gath",
                             addr_space=gather_addr_space)
            nc.gpsimd.dma_start(bounce[:], esh_d[:])
            nc.gpsimd.collective_compute(
                "AllGather",
                ALU.bypass,
                replica_groups=[list(range(NCORES))],
                ins=[bounce.opt()],
                outs=[gath.opt()],
            )

            # ---- local anchor-side prep (overlaps the collective)
            esh_sb = sb.tile([D, NA], F16, tag="esh_sb")
            nc.sync.dma_start(esh_sb[:], esh_d[:])
            amask_sb = sb.tile([128, NCLASS], F32, tag="amask_sb")
            nc.sync.dma_start(amask_sb[:], amask_d[:])

            an2 = sb.tile([D, NA], F16, tag="an2")
            nc.vector.tensor_scalar_mul(an2[:], esh_sb[:], 2.0)

            ones = sb.tile([128, 1], F32, tag="ones")
            nc.vector.memset(ones[:], 1.0)

            # squared norms: square, then partition-reduce via ones-matmul
            sqaf = sb.tile([128, NA], F32, tag="sqaf")
            nc.scalar.activation(sqaf[:], esh_sb[:], AFT.Square)
            sqa_row = sb.tile([1, NA], F32, tag="sqa_row")
            sq_row = sb.tile([1, BPAD], F32, tag="sq_row")
            eb = sb.tile([128, NCLASS, TPC, 128], F16, tag="eb")
            sqf = sb.tile([128, NCLASS, TPC, 128], F32, tag="sqf")
            with tc.tile_pool(name="ps_sq", bufs=2, space=bass.MemorySpace.PSUM) as ps_sq:
                for h, w in ((0, 512), (512, 512), (1024, 256)):
                    pt = ps_sq.tile([1, 512], F32, tag="pt")
                    nc.tensor.matmul(pt[0:1, 0:w], ones[:], sqaf[:, h:h + w],
                                     start=True, stop=True)
                    nc.scalar.copy(sqa_row[0:1, h:h + w], pt[0:1, 0:w])

                # full matrix: gath[(t d), (k q)] -> eb[d, k, t, q]
                # (core t's shard holds tile t of every class k)
                nc.sync.dma_start(
                    eb[:], gath[:].rearrange("(t d) (k q) -> d k t q", d=128, k=NCLASS)
                )
                nc.scalar.activation(sqf[:], eb[:], AFT.Square)
                for k in range(NCLASS):
                    for t0 in (0, 4):
                        pt = ps_sq.tile([1, 512], F32, tag="pt")
                        nc.tensor.matmul(pt[0:1, :], ones[:], sqf[:, k, t0:t0 + 4, :],
                                         start=True, stop=True)
                        nc.scalar.copy(
                            sq_row[0:1, k * SEG + t0 * 128: k * SEG + t0 * 128 + 512],
                            pt[0:1, :])

            # reshape the [1, N] rows to per-partition layout via a DRAM bounce
            scr_a = dram.tile([1, NA], F32, tag="scr_a")
            scr_j = dram.tile([1, BPAD], F32, tag="scr_j")
            nc.sync.dma_start(scr_a[:], sqa_row[:])
            nc.sync.dma_start(scr_j[:], sq_row[:])
            sqa_pk = sb.tile([128, NCLASS], F32, tag="sqa_pk")
            nc.sync.dma_start(sqa_pk[:], scr_a[:].rearrange("a (k q) -> (a q) k", q=128))
            sqv = sb.tile([128, NJT], F32, tag="sqv")
            nc.sync.dma_start(sqv[:], scr_j[:].rearrange("a (t q) -> (a q) t", q=128))

            # ---- main sweep over 80 j-tiles, accumulating in 2*dot - sq_j space
            acc_hn = sb.tile([128, NA], F32, tag="acc_hn")
            acc_hp = sb.tile([128, NA], F32, tag="acc_hp")
            nc.vector.memset(acc_hn[:], -3.0e38)
            nc.vector.memset(acc_hp[:], 3.0e38)

            with tc.tile_pool(name="ps_g", bufs=2, space=bass.MemorySpace.PSUM) as ps_g:
                for t in range(NJT):
                    c, ri = t // TPC, t % TPC
                    nreal = min(max(int(R[c]) - ri * 128, 0), 128)
                    if nreal == 0:
                        # pure padding rows: can never win a hardest-neg/pos
                        continue
                    g = ps_g.tile([128, NA], F32, tag="g")
                    for h, w in ((0, 512), (512, 512), (1024, 256)):
                        nc.tensor.matmul(g[:, h:h + w], eb[:, c, ri, :],
                                         an2[:, h:h + w], start=True, stop=True)
                    sq_t = sqv[:, t:t + 1]
                    # hardest-negative: all anchor columns except own class c
                    if c > 0:
                        nc.vector.scalar_tensor_tensor(
                            acc_hn[:, 0:c * 128], g[:, 0:c * 128], sq_t,
                            acc_hn[:, 0:c * 128], op0=ALU.subtract, op1=ALU.max)
                    if c < NCLASS - 1:
                        nc.vector.scalar_tensor_tensor(
                            acc_hn[:, (c + 1) * 128:NA], g[:, (c + 1) * 128:NA], sq_t,
                            acc_hn[:, (c + 1) * 128:NA], op0=ALU.subtract, op1=ALU.max)
                    # hardest-positive: own-class columns, real j rows only
                    sl = slice(c * 128, (c + 1) * 128)
                    nc.vector.scalar_tensor_tensor(
                        acc_hp[0:nreal, sl], g[0:nreal, sl], sqv[0:nreal, t:t + 1],
                        acc_hp[0:nreal, sl], op0=ALU.subtract, op1=ALU.min)

            # ---- fold on chip: transpose-reduce over j-partials, loss math,
            # and a final partition sum down to [1, 1]
            ident = sb.tile([128, 128], F32, tag="ident")
            masks.make_identity(nc, ident[:])
            hn_t = sb.tile([128, NCLASS], F32, tag="hn_t")
            hp_t = sb.tile([128, NCLASS], F32, tag="hp_t")
            with tc.tile_pool(name="ps_f", bufs=2, space=bass.MemorySpace.PSUM) as ps_f:
                for b in range(NCLASS):
                    pn = ps_f.tile([128, 128], F32, tag="pn")
                    nc.tensor.transpose(pn[:], acc_hn[:, b * 128:(b + 1) * 128], ident[:])
                    nc.vector.reduce_max(hn_t[:, b:b + 1], pn[:], axis=mybir.AxisListType.X)
                    pp = ps_f.tile([128, 128], F32, tag="pp")
                    nc.tensor.transpose(pp[:], acc_hp[:, b * 128:(b + 1) * 128], ident[:])
                    nc.vector.tensor_reduce(hp_t[:, b:b + 1], pp[:], op=ALU.min,
                                            axis=mybir.AxisListType.X)

                hn2 = sb.tile([128, NCLASS], F32, tag="hn2")
                nc.vector.tensor_tensor(hn2[:], sqa_pk[:], hn_t[:], op=ALU.subtract)
                nc.vector.tensor_scalar_max(hn2[:], hn2[:], 0.0)
                nc.scalar.sqrt(hn2[:], hn2[:])
                hp2 = sb.tile([128, NCLASS], F32, tag="hp2")
                nc.vector.tensor_tensor(hp2[:], sqa_pk[:], hp_t[:], op=ALU.subtract)
                nc.vector.tensor_scalar_max(hp2[:], hp2[:], 0.0)
                nc.scalar.sqrt(hp2[:], hp2[:])

                li = sb.tile([128, NCLASS], F32, tag="li")
                nc.vector.tensor_tensor(li[:], hp2[:], hn2[:], op=ALU.subtract)
                nc.vector.tensor_scalar(li[:], li[:], float(MARGIN), 0.0,
                                        op0=ALU.add, op1=ALU.max)
                nc.vector.tensor_tensor(li[:], li[:], amask_sb[:], op=ALU.mult)
                li1 = sb.tile([128, 1], F32, tag="li1")
                nc.vector.reduce_sum(li1[:], li[:], axis=mybir.AxisListType.X)
                pl = ps_f.tile([1, 1], F32, tag="pl")
                nc.tensor.matmul(pl[0:1, 0:1], li1[:], ones[:], start=True, stop=True)
                loss_sb = sb.tile([1, 1], F32, tag="loss_sb")
                nc.scalar.copy(loss_sb[:], pl[0:1, 0:1])
            nc.sync.dma_start(loss_d[:], loss_sb[:])
    nc.compile()
    return nc


def prepare(embeddings, labels):
    emb = np.ascontiguousarray(np.asarray(embeddings, dtype=np.float32))
    lab = np.asarray(labels).astype(np.int64).ravel()
    assert emb.shape == (B, D)
    order = np.argsort(lab, kind="stable")
    es = emb[order]
    counts = np.bincount(lab, minlength=NCLASS)
    assert counts.max() <= SEG, counts
    ep = np.zeros((BPAD, D), np.float32)
    ep[:, 0] = DUMMY_VAL
    ofs = np.concatenate([[0], np.cumsum(counts)])
    for c in range(NCLASS):
        ep[c * SEG: c * SEG + counts[c]] = es[ofs[c]:ofs[c + 1]]
    eph = ep.astype(np.float16)
    q = np.arange(128)
    in_maps = []
    for i in range(NCORES):
        rows = np.concatenate(
            [eph[k * SEG + i * 128: k * SEG + (i + 1) * 128] for k in range(NCLASS)], 0
        )
        esh = np.ascontiguousarray(rows.T)                       # [128, 1280] f16
        amask = (i * 128 + q[:, None] < counts[None, :]).astype(np.float32)
        in_maps.append({"esh": esh, "amask": amask})
    return in_maps, counts


def combine(results, counts=None, in_maps=None):
    total = 0.0
    for i in range(NCORES):
        total += float(np.asarray(results[i]["loss"], np.float32)[0, 0])
    return np.asarray(total / B, dtype=np.float32)


def kernel(embeddings, labels, _trace=False, _tmpdir=None):
    in_maps, counts = prepare(embeddings, labels)
    nc = build_nc(list(counts))
    res = run_bass_kernel_spmd(
        nc, in_maps, list(range(NCORES)), trace=_trace, tmpdir=_tmpdir
    )
    out = combine(res.results)
    if _trace:
        return out, res
    return out



# revision 7
# speedup vs baseline: 1.3780x; 1.3780x over previous
"""BatchHardTripletLoss on 8 TRN2 NeuronCores (Bass/Tile).

The warm-path cost of this problem is host<->device traffic over the axon
tunnel (fixed RTT ~40ms + ~19ms/MB measured), not on-chip compute, so the
kernel is built around moving as few bytes as possible:

  - Host: sort rows by label, pad every class segment to SEG=1024 rows with
    far-away dummy rows (first coord DUMMY_VAL=192 -> sq ~ 3.7e4, so dummies
    never win a hardest-negative).  Core i uploads ONLY its anchor shard:
    tile i of every class, transposed, in fp8-e4m3 ([128, 1280] = 160 KB),
    plus a 5 KB anchor-validity mask.  Total upload ~1.36 MB.
  - On chip: an AllGather over NeuronLink reassembles the full padded fp8
    embedding matrix (the 8 anchor shards tile it exactly).  Halved squared
    norms are computed on chip (square + ones-matmul partition reduction)
    and exchanged with a second, tiny AllGather ([1,1280] f32 -> [8,1280]).
    The distance sweep matmuls fp8 esh directly (no 2x pre-scale pass) and
    accumulates hardest pos/neg in "m = dot - sq_j/2" space via DVE
    scalar_tensor_tensor off PSUM:
        min dist^2 = 2*(sq_a/2 - max_j m),  max dist^2 = 2*(sq_a/2 - min_j m)
    so the final per-anchor loss is relu(sqrt2*(sqrt(u_p) - sqrt(u_n)) + 1).
    A PE-transpose fold reduces to one f32 loss partial per core ([1,1]).
  - Host folds 8 scalars.

fp8-e4m3 quantization of the embeddings moves the final loss by ~7.5e-4
relative (tolerance 2e-2): distance errors average out over D=128 dims.
"""

import numpy as np
import ml_dtypes

import jax

# Cache compiled XLA executables on disk: run_bass_kernel_spmd builds a fresh
# jax.jit closure per call, so without this every warm call pays a ~165 ms
# recompile before dispatch.
try:
    jax.config.update("jax_compilation_cache_dir", "/tmp/jax_comp_cache")
    jax.config.update("jax_persistent_cache_min_compile_time_secs", 0.0)
    jax.config.update("jax_persistent_cache_min_entry_size_bytes", 0)
except Exception:
    pass

import concourse.bass as bass
import concourse.bacc as bacc
import concourse.tile as tile
from concourse import masks, mybir
from concourse.bass_utils import run_bass_kernel_spmd

B, D, NCLASS = 8192, 128, 10
SEG = 1024                 # padded rows per class
TPC = SEG // 128           # 128-row tiles per class = 8
NCORES = 8
BPAD = NCLASS * SEG        # 10240
NJT = BPAD // 128          # 80 j-tiles
NA = NCLASS * 128          # anchors per core = 1280
F32 = mybir.dt.float32
F8 = mybir.dt.float8e4
NP_F8 = ml_dtypes.float8_e4m3
AFT = mybir.ActivationFunctionType
ALU = mybir.AluOpType
MARGIN = 1.0
DUMMY_VAL = 192.0          # exactly representable in e4m3; 192^2 >> any real dist^2
SQRT2 = 1.4142135623730951


def build_nc(R, gather_addr_space="Shared"):
    """R: real row count per class (0 <= R[k] <= SEG)."""
    nc = bacc.Bacc()
    esh_d = nc.dram_tensor("esh", [D, NA], F8, kind="ExternalInput")
    amask_d = nc.dram_tensor("amask", [128, NCLASS], F32, kind="ExternalInput")
    loss_d = nc.dram_tensor("loss", [1, 1], F32, kind="ExternalOutput")

    with tile.TileContext(nc) as tc:
        with (
            tc.tile_pool(name="sb", bufs=1) as sb,
            tc.tile_pool(name="dram", bufs=1, space="DRAM") as dram,
        ):
            # ---- kick off the AllGather first: cores exchange anchor shards
            # so each one can rebuild the full [128, 10240] embedding matrix.
            bounce = dram.tile([D, NA], F8, tag="bounce")
            gath = dram.tile([NCORES * D, NA], F8, tag="gath",
                             addr_space=gather_addr_space)
            nc.gpsimd.dma_start(bounce[:], esh_d[:])
            nc.gpsimd.collective_compute(
                "AllGather",
                ALU.bypass,
                replica_groups=[list(range(NCORES))],
                ins=[bounce.opt()],
                outs=[gath.opt()],
            )

            # ---- local anchor-side prep (overlaps the collective)
            esh_sb = sb.tile([D, NA], F8, tag="esh_sb")
            nc.sync.dma_start(esh_sb[:], esh_d[:])
            amask_sb = sb.tile([128, NCLASS], F32, tag="amask_sb")
            nc.sync.dma_start(amask_sb[:], amask_d[:])

            ones = sb.tile([128, 1], F32, tag="ones")
            nc.vector.memset(ones[:], 1.0)

            # halved squared norms of own anchors: square, partition-reduce
            # via ones-matmul, fuse the *0.5 into the PSUM->SBUF copy.
            sqaf = sb.tile([128, NA], F32, tag="sqaf")
            nc.scalar.activation(sqaf[:], esh_sb[:], AFT.Square)
            sqh_row = sb.tile([1, NA], F32, tag="sqh_row")
            with tc.tile_pool(name="ps_sq", bufs=2, space=bass.MemorySpace.PSUM) as ps_sq:
                for h, w in ((0, 512), (512, 512), (1024, 256)):
                    pt = ps_sq.tile([1, 512], F32, tag="pt")
                    nc.tensor.matmul(pt[0:1, 0:w], ones[:], sqaf[:, h:h + w],
                                     start=True, stop=True)
                    nc.scalar.mul(sqh_row[0:1, h:h + w], pt[0:1, 0:w], 0.5)

            # reshape the [1, NA] row to per-partition layout via a DRAM bounce:
            # sqa_pk[p, k] = sq/2 of own anchor (k*128+p)
            scr_a = dram.tile([1, NA], F32, tag="scr_a")
            nc.sync.dma_start(scr_a[:], sqh_row[:])
            sqa_pk = sb.tile([128, NCLASS], F32, tag="sqa_pk")
            nc.sync.dma_start(sqa_pk[:], scr_a[:].rearrange("a (k q) -> (a q) k", q=128))

            # ---- second AllGather: exchange the halved norms (5 KB) in
            # per-partition layout, so every core has sq_j/2 for all rows:
            # gath2[(s p), k] = sq/2 of class-k tile-s row p.
            bounce2 = dram.tile([128, NCLASS], F32, tag="bounce2")
            gath2 = dram.tile([NCORES * 128, NCLASS], F32, tag="gath2",
                              addr_space=gather_addr_space)
            nc.sync.dma_start(bounce2[:], sqa_pk[:])
            nc.gpsimd.collective_compute(
                "AllGather",
                ALU.bypass,
                replica_groups=[list(range(NCORES))],
                ins=[bounce2.opt()],
                outs=[gath2.opt()],
            )

            # full matrix: gath[(t d), (k q)] -> eb[d, k, t, q]
            # (core t's shard holds tile t of every class k)
            eb = sb.tile([128, NCLASS, TPC, 128], F8, tag="eb")
            nc.sync.dma_start(
                eb[:], gath[:].rearrange("(t d) (k q) -> d k t q", d=128, k=NCLASS)
            )

            # sqv[p, s, k] = sq/2 of j-row p of tile s of class k
            sqv = sb.tile([128, TPC, NCLASS], F32, tag="sqv")
            nc.sync.dma_start(sqv[:], gath2[:].rearrange("(s p) k -> p s k", p=128))

            # ---- main sweep over 80 j-tiles, accumulating m = dot - sq_j/2
            acc_hn = sb.tile([128, NA], F32, tag="acc_hn")
            acc_hp = sb.tile([128, NA], F32, tag="acc_hp")
            nc.vector.memset(acc_hn[:], -3.0e38)
            nc.vector.memset(acc_hp[:], 3.0e38)

            with tc.tile_pool(name="ps_g", bufs=2, space=bass.MemorySpace.PSUM) as ps_g:
                for t in range(NJT):
                    c, ri = t // TPC, t % TPC
                    nreal = min(max(int(R[c]) - ri * 128, 0), 128)
                    if nreal == 0:
                        # pure padding rows: can never win a hardest-neg/pos
                        continue
                    g = ps_g.tile([128, NA], F32, tag="g")
                    for h, w in ((0, 512), (512, 512), (1024, 256)):
                        nc.tensor.matmul(g[:, h:h + w], eb[:, c, ri, :],
                                         esh_sb[:, h:h + w], start=True, stop=True)
                    sq_t = sqv[:, ri, c:c + 1]
                    # hardest-negative: all anchor columns except own class c
                    if c > 0:
                        nc.vector.scalar_tensor_tensor(
                            acc_hn[:, 0:c * 128], g[:, 0:c * 128], sq_t,
                            acc_hn[:, 0:c * 128], op0=ALU.subtract, op1=ALU.max)
                    if c < NCLASS - 1:
                        nc.vector.scalar_tensor_tensor(
                            acc_hn[:, (c + 1) * 128:NA], g[:, (c + 1) * 128:NA], sq_t,
                            acc_hn[:, (c + 1) * 128:NA], op0=ALU.subtract, op1=ALU.max)
                    # hardest-positive: own-class columns, real j rows only
                    sl = slice(c * 128, (c + 1) * 128)
                    nc.vector.scalar_tensor_tensor(
                        acc_hp[0:nreal, sl], g[0:nreal, sl], sqv[0:nreal, ri, c:c + 1],
                        acc_hp[0:nreal, sl], op0=ALU.subtract, op1=ALU.min)

            # ---- fold on chip: transpose-reduce over j-partials, loss math,
            # and a final partition sum down to [1, 1]
            ident = sb.tile([128, 128], F32, tag="ident")
            masks.make_identity(nc, ident[:])
            hn_t = sb.tile([128, NCLASS], F32, tag="hn_t")
            hp_t = sb.tile([128, NCLASS], F32, tag="hp_t")
            with tc.tile_pool(name="ps_f", bufs=2, space=bass.MemorySpace.PSUM) as ps_f:
                for b in range(NCLASS):
                    pn = ps_f.tile([128, 128], F32, tag="pn")
                    nc.tensor.transpose(pn[:], acc_hn[:, b * 128:(b + 1) * 128], ident[:])
                    nc.vector.reduce_max(hn_t[:, b:b + 1], pn[:], axis=mybir.AxisListType.X)
                    pp = ps_f.tile([128, 128], F32, tag="pp")
                    nc.tensor.transpose(pp[:], acc_hp[:, b * 128:(b + 1) * 128], ident[:])
                    nc.vector.tensor_reduce(hp_t[:, b:b + 1], pp[:], op=ALU.min,
                                            axis=mybir.AxisListType.X)

                # u_n = sq_a/2 - max_j m  (= min dist^2 / 2);  u_p likewise
                hn2 = sb.tile([128, NCLASS], F32, tag="hn2")
                nc.vector.tensor_tensor(hn2[:], sqa_pk[:], hn_t[:], op=ALU.subtract)
                nc.vector.tensor_scalar_max(hn2[:], hn2[:], 0.0)
                nc.scalar.sqrt(hn2[:], hn2[:])
                hp2 = sb.tile([128, NCLASS], F32, tag="hp2")
                nc.vector.tensor_tensor(hp2[:], sqa_pk[:], hp_t[:], op=ALU.subtract)
                nc.vector.tensor_scalar_max(hp2[:], hp2[:], 0.0)
                nc.scalar.sqrt(hp2[:], hp2[:])

                # loss_i = relu(sqrt2*(sqrt(u_p) - sqrt(u_n)) + margin) * amask
                li = sb.tile([128, NCLASS], F32, tag="li")
                nc.vector.tensor_tensor(li[:], hp2[:], hn2[:], op=ALU.subtract)
                nc.vector.tensor_scalar(li[:], li[:], SQRT2, float(MARGIN),
                                        op0=ALU.mult, op1=ALU.add)
                zcol = sb.tile([128, 1], F32, tag="zcol")
                nc.vector.memset(zcol[:], 0.0)
                nc.vector.scalar_tensor_tensor(li[:], li[:], zcol[:], amask_sb[:],
                                               op0=ALU.max, op1=ALU.mult)
                li1 = sb.tile([128, 1], F32, tag="li1")
                nc.vector.reduce_sum(li1[:], li[:], axis=mybir.AxisListType.X)
                pl = ps_f.tile([1, 1], F32, tag="pl")
                nc.tensor.matmul(pl[0:1, 0:1], li1[:], ones[:], start=True, stop=True)
                loss_sb = sb.tile([1, 1], F32, tag="loss_sb")
                nc.scalar.copy(loss_sb[:], pl[0:1, 0:1])
            nc.sync.dma_start(loss_d[:], loss_sb[:])
    nc.compile()
    return nc


_NC_CACHE: dict = {}


def get_nc(counts):
    key = tuple(int(c) for c in counts)
    nc = _NC_CACHE.get(key)
    if nc is None:
        nc = build_nc(list(key))
        _NC_CACHE[key] = nc
    return nc


def prepare(embeddings, labels):
    emb = np.ascontiguousarray(np.asarray(embeddings, dtype=np.float32))
    lab = np.asarray(labels).astype(np.int64).ravel()
    assert emb.shape == (B, D)
    order = np.argsort(lab, kind="stable")
    es = emb[order]
    counts = np.bincount(lab, minlength=NCLASS)
    assert counts.max() <= SEG, counts
    ep = np.zeros((BPAD, D), np.float32)
    ep[:, 0] = DUMMY_VAL
    ofs = np.concatenate([[0], np.cumsum(counts)])
    for c in range(NCLASS):
        ep[c * SEG: c * SEG + counts[c]] = es[ofs[c]:ofs[c + 1]]
    eph = ep.astype(NP_F8)
    # core i's shard: tile i of every class, transposed -> [128, 1280] fp8
    epr = eph.reshape(NCLASS, TPC, 128, D)
    q = np.arange(128)
    in_maps = []
    for i in range(NCORES):
        esh = np.ascontiguousarray(
            epr[:, i].reshape(NCLASS * 128, D).T)              # [128, 1280] f8
        amask = (i * 128 + q[:, None] < counts[None, :]).astype(np.float32)
        in_maps.append({"esh": esh, "amask": amask})
    return in_maps, counts


def combine(results, counts=None, in_maps=None):
    total = 0.0
    for i in range(NCORES):
        total += float(np.asarray(results[i]["loss"], np.float32)[0, 0])
    return np.asarray(total / B, dtype=np.float32)


def kernel(embeddings, labels, _trace=False, _tmpdir=None):
    in_maps, counts = prepare(embeddings, labels)
    nc = get_nc(counts)
    res = run_bass_kernel_spmd(
        nc, in_maps, list(range(NCORES)), trace=_trace, tmpdir=_tmpdir
    )
    out = combine(res.results)
    if _trace:
        return out, res
    return out
